# revision 1
# baseline (speedup 1.0000x reference)
"""AttentiveConv TRN2 kernel: out = (softmax_n((text@We)@ctx^T) @ ctx) @ W2^T.

Sharded data-parallel over batch B=8 across 8 NeuronCores (one batch each).
Inputs are pre-transposed / fp32r-pre-rounded on host; matmuls run in fp32r
(TF32, 1 cyc/row — 4x faster than fp32) except mm3 which runs in bf16 (the
output error is dominated by the scores path, so bf16 attn costs nothing).

Per-core dataflow (PSUM accumulates fp32 throughout):
  A: tempT[D,N]   = matmul(lhsT=We[d',d],      rhs=textT[d',n])       fp32r
  B: scoresT[M,N] = matmul(lhsT=ctxT[d',m],    rhs=tempT[d',n])       fp32r
     softmax along the free axis n per 128-row m-tile:
     attn = exp(s - max_n)/Z (exp+Z in one ACT pass), attn -> bf16 DRAM spill
  C: resT[D,N]    = matmul(lhsT=ctx[m,d'],     rhs=attnT[m,n])        bf16
  D: out[N,D]     = matmul(lhsT=resT[d',n],    rhs=W2T[d',d])         fp32r

scoresT layout (scores transposed) makes the softmax axis the free axis and
every matmul consume its predecessor's natural output layout. attn round-trips
through DRAM because softmax-over-queries prevents flash-style fusion (the
softmax axis N differs from mm3's contraction axis M) and full attn (16MB
fp32r / 8MB bf16) cannot stay in SBUF next to ctx/tempT.

Measured: ~335-365 us/core steady-state on HW (PE roofline 327 us; 25.8
GFLOP/core at 78.6 TFLOP/s); relative error vs fp32 reference 3.6e-3.
"""

import sys

sys.path.insert(0, "/opt/trn_rl_repo")

from contextlib import ExitStack

import ml_dtypes
import numpy as np

B, N, M, D = 8, 2048, 2048, 1024
P = 128
KT = D // P  # 8 contraction tiles for d'
MT = M // P  # 16 m-tiles
NCH_A = 256  # phase A n-chunk
NCH_C = 256  # phase C/D n-chunk
SPLIT_WE = True  # split initial We load per output-column block
SPLIT_C_LOADS = True  # per-mt attn loads in phase C
PSA_BUFS = 4
PSB_BUFS = 8
CTP_BUFS = 4
EPL_BUFS = 2
PSC_BUFS = 6
TXP_BUFS = 2
ATP_BUFS = 2
ACP_BUFS = 3
KEEP_ATTN = False
BIG_PSUM_B = True  # one 4-bank [128,2048] psum tile per m-tile in phase B
D_EVICT_ACT = False  # route phase D psum evictions to ScalarE (ACT idle in C/D)  # SBUF-kept attn tiles hurt replica pipelining; DRAM RT is fine

_cache = {}


def r11(x: np.ndarray) -> np.ndarray:
    """Round fp32 to fp32r (TF32: 11 explicit mantissa bits, round-nearest-even)."""
    x = np.ascontiguousarray(x, dtype=np.float32)
    u = x.view(np.uint32).astype(np.uint64)
    bias = ((u >> 12) & 1) + 0x7FF
    u = (u + bias) & np.uint64(0xFFFFF000)
    return u.astype(np.uint32).view(np.float32).reshape(x.shape)


def _build(replicas=1, phases="ABCD"):
    """replicas>1 repeats the whole pipeline in one NEFF (for HW timing
    amortization); phases subsets the pipeline (for attribution)."""
    import concourse.bass as bass  # noqa: F401
    import concourse.mybir as mybir
    import concourse.tile as tile
    from concourse import bacc

    f32 = mybir.dt.float32
    f32r = mybir.dt.float32r

    nc = bacc.Bacc(None, target_bir_lowering=False)

    textT_d = nc.declare_dram_parameter("textT", [D, N], f32r, isOutput=False)
    ctxT_d = nc.declare_dram_parameter("ctxT", [D, M], f32r, isOutput=False)
    ctx_d = nc.declare_dram_parameter("ctx", [M, D], mybir.dt.bfloat16, isOutput=False)
    we_d = nc.declare_dram_parameter("we", [D, D], f32r, isOutput=False)
    w2T_d = nc.declare_dram_parameter("w2T", [D, D], f32r, isOutput=False)
    out_d = nc.declare_dram_parameter("out", [N, D], f32, isOutput=True)
    attn_sc = nc.dram_tensor("attn_sc", [MT, P, N], mybir.dt.bfloat16)

    with tile.TileContext(nc) as tc, ExitStack() as top:
        # whole-kernel residents
        consts = top.enter_context(tc.tile_pool(name="consts", bufs=1))
        ctx_sb = consts.tile([P, MT, D], mybir.dt.bfloat16)  # 32KB/p, phase C lhsT (bf16)
        wslab = consts.tile([P, KT, D], f32r)  # 32KB/p: We during A, W2T during D

        for _rep in range(replicas):
            _emit_pipeline(
                nc, tc, mybir, f32, f32r, phases, ctx_sb, wslab,
                textT_d, ctxT_d, ctx_d, we_d, w2T_d, out_d, attn_sc,
            )

    nc.compile()
    return nc


def _emit_pipeline(
    nc, tc, mybir, f32, f32r, phases, ctx_sb, wslab,
    textT_d, ctxT_d, ctx_d, we_d, w2T_d, out_d, attn_sc,
):
    from contextlib import ExitStack

    with ExitStack() as rep_stack:
        we_ap = we_d[:].rearrange("(kt p) d -> p kt d", p=P)
        keep_attn = {}
        # attn tiles for the last ATP_BUFS m-tiles stay resident into phase C
        # (skips their DRAM round-trip on the critical path)
        atp = rep_stack.enter_context(tc.tile_pool(name="atp", bufs=ATP_BUFS))

        with ExitStack() as ab_stack:
            tempT_pool = ab_stack.enter_context(tc.tile_pool(name="tempT", bufs=1))
            tempT = tempT_pool.tile([P, KT, N], f32r)  # 64KB/p

            # ---- Phase A: tempT = We.T-layout matmul over textT ----
            with ExitStack() as a_stack:
              if "A" in phases:
                txp = a_stack.enter_context(tc.tile_pool(name="txp", bufs=TXP_BUFS))
                psA = a_stack.enter_context(
                    tc.tile_pool(name="psA", bufs=PSA_BUFS, space="PSUM")
                )
                textT_ap = textT_d[:].rearrange("(kt p) n -> p kt n", p=P)
                for ch in range(N // NCH_A):
                    tx = txp.tile([P, KT, NCH_A], f32r)
                    nc.sync.dma_start(
                        tx[:], textT_ap[:, :, ch * NCH_A : (ch + 1) * NCH_A]
                    )
                    if ch == 0:
                        # split We load per output-column block so dt=0's
                        # matmuls start after 1/8th of the weight traffic
                        if SPLIT_WE:
                            for dt in range(KT):
                                nc.sync.dma_start(
                                    wslab[:, :, dt * P : (dt + 1) * P],
                                    we_ap[:, :, dt * P : (dt + 1) * P],
                                )
                        else:
                            nc.sync.dma_start(wslab[:], we_ap[:])
                    for dt in range(KT):
                        ps = psA.tile([P, NCH_A], f32)
                        for kt in range(KT):
                            nc.tensor.matmul(
                                ps[:],
                                wslab[:, kt, dt * P : (dt + 1) * P],
                                tx[:, kt],
                                start=(kt == 0),
                                stop=(kt == KT - 1),
                            )
                        nc.vector.tensor_copy(
                            tempT[:, dt, ch * NCH_A : (ch + 1) * NCH_A], ps[:]
                        )

            # phase C/D weights stream in during B
            if "C" in phases:
                nc.sync.dma_start(
                    ctx_sb[:], ctx_d[:].rearrange("(mt p) d -> p mt d", p=P)
                )
            if "D" in phases:
                nc.sync.dma_start(
                    wslab[:], w2T_d[:].rearrange("(kt p) d -> p kt d", p=P)
                )

            # ---- Phase B: scoresT per m-tile + softmax over n, spill attn ----
            with ExitStack() as b_stack:
              if "B" in phases:
                ctp = b_stack.enter_context(tc.tile_pool(name="ctp", bufs=CTP_BUFS))
                psB = b_stack.enter_context(
                    tc.tile_pool(
                        name="psB",
                        bufs=2 if BIG_PSUM_B else PSB_BUFS,
                        space="PSUM",
                    )
                )
                smp = b_stack.enter_context(tc.tile_pool(name="smp", bufs=4))
                epl = b_stack.enter_context(tc.tile_pool(name="epl", bufs=EPL_BUFS))
                ctxT_ap = ctxT_d[:].rearrange("(kt p) m -> p kt m", p=P)
                NJ = 512
                for mt in range(MT):
                    ctm = ctp.tile([P, KT, P], f32r)
                    nc.sync.dma_start(ctm[:], ctxT_ap[:, :, mt * P : (mt + 1) * P])
                    if BIG_PSUM_B:
                        psw = psB.tile([P, N], f32, tag="psB", name="psB")
                        for j in range(N // NJ):
                            for kt in range(KT):
                                nc.tensor.matmul(
                                    psw[:, j * NJ : (j + 1) * NJ],
                                    ctm[:, kt],
                                    tempT[:, kt, j * NJ : (j + 1) * NJ],
                                    start=(kt == 0),
                                    stop=(kt == KT - 1),
                                )
                        nmax = smp.tile([P, 1], f32)
                        nc.vector.reduce_max(
                            nmax[:], psw[:], axis=mybir.AxisListType.X, negate=True
                        )
                        attn = atp.tile([P, N], mybir.dt.bfloat16)
                        esb = epl.tile([P, N], f32)
                        z = smp.tile([P, 1], f32)
                        nc.scalar.activation(
                            esb[:],
                            psw[:],
                            mybir.ActivationFunctionType.Exp,
                            bias=nmax[:],
                            accum_out=z[:],
                        )
                        zinv = smp.tile([P, 1], f32)
                        nc.vector.reciprocal(zinv[:], z[:])
                        nc.vector.tensor_scalar_mul(attn[:], esb[:], zinv[:])
                        keep_attn[mt] = attn
                        if not KEEP_ATTN or mt < MT - ATP_BUFS:
                            nc.sync.dma_start(attn_sc[mt], attn[:])
                        continue
                    pss = []
                    for j in range(N // NJ):
                        ps = psB.tile([P, NJ], f32, tag="psBs", name="psBs")
                        for kt in range(KT):
                            nc.tensor.matmul(
                                ps[:],
                                ctm[:, kt],
                                tempT[:, kt, j * NJ : (j + 1) * NJ],
                                start=(kt == 0),
                                stop=(kt == KT - 1),
                            )
                        pss.append(ps)
                    nm4 = smp.tile([P, 4], f32)
                    for j, ps in enumerate(pss):
                        nc.vector.reduce_max(
                            nm4[:, j : j + 1], ps[:], axis=mybir.AxisListType.X
                        )
                    nmax = smp.tile([P, 1], f32)
                    nc.vector.reduce_max(
                        nmax[:], nm4[:], axis=mybir.AxisListType.X, negate=True
                    )
                    attn = atp.tile([P, N], mybir.dt.bfloat16)
                    esb = epl.tile([P, N], f32)
                    zp4 = smp.tile([P, 4], f32)
                    for j, ps in enumerate(pss):
                        nc.scalar.activation(
                            esb[:, j * NJ : (j + 1) * NJ],
                            ps[:],
                            mybir.ActivationFunctionType.Exp,
                            bias=nmax[:],
                            accum_out=zp4[:, j : j + 1],
                        )
                    z = smp.tile([P, 1], f32)
                    nc.vector.reduce_sum(z[:], zp4[:], axis=mybir.AxisListType.X)
                    zinv = smp.tile([P, 1], f32)
                    nc.vector.reciprocal(zinv[:], z[:])
                    nc.vector.tensor_scalar_mul(attn[:], esb[:], zinv[:])
                    keep_attn[mt] = attn
                    if not KEEP_ATTN or mt < MT - ATP_BUFS:
                        nc.sync.dma_start(attn_sc[mt], attn[:])

        # ---- Phase C+D: resT accum over m, then out = resT.T @ W2T ----
        with ExitStack() as cd_stack:
          if "C" in phases:
            acp = cd_stack.enter_context(tc.tile_pool(name="acp", bufs=ACP_BUFS))
            rtp = cd_stack.enter_context(tc.tile_pool(name="rtp", bufs=2))
            outp = cd_stack.enter_context(tc.tile_pool(name="outp", bufs=2))
            psC = cd_stack.enter_context(tc.tile_pool(name="psC", bufs=PSC_BUFS, space="PSUM"))
            psD = cd_stack.enter_context(tc.tile_pool(name="psD", bufs=2, space="PSUM"))
            attn_ap = attn_sc[:].rearrange("mt p n -> p mt n")
            for ch in range(N // NCH_C):
                ach = acp.tile([P, MT, NCH_C], mybir.dt.bfloat16)
                # per-mt loads so C's accumulation chases B's per-mt attn
                # writes instead of waiting for all of B to finish
                n_dram_mt = MT - ATP_BUFS if KEEP_ATTN else MT
                if SPLIT_C_LOADS:
                    for mt in range(n_dram_mt):
                        nc.sync.dma_start(
                            ach[:, mt],
                            attn_ap[:, mt, ch * NCH_C : (ch + 1) * NCH_C],
                        )
                else:
                    nc.sync.dma_start(
                        ach[:, :n_dram_mt],
                        attn_ap[:, :n_dram_mt, ch * NCH_C : (ch + 1) * NCH_C],
                    )
                resT = rtp.tile([P, KT, NCH_C], f32r)
                for g in range(2):
                    pss = []
                    for i in range(4):
                        pst = psC.tile([P, NCH_C], f32, tag="psC", name="psC")
                        pss.append(pst)
                    for mt in range(MT):
                        if KEEP_ATTN and mt >= MT - ATP_BUFS:
                            rhs = keep_attn[mt][:, ch * NCH_C : (ch + 1) * NCH_C]
                        else:
                            rhs = ach[:, mt]
                        for i in range(4):
                            dtt = g * 4 + i
                            nc.tensor.matmul(
                                pss[i][:],
                                ctx_sb[:, mt, dtt * P : (dtt + 1) * P],
                                rhs,
                                start=(mt == 0),
                                stop=(mt == MT - 1),
                            )
                    for i in range(4):
                        nc.vector.tensor_copy(resT[:, g * 4 + i], pss[i][:])
                for nb in range(NCH_C // P if "D" in phases else 0):
                    osb = outp.tile([P, D], f32)
                    for dc in range(2):
                        ps = psD.tile([P, 512], f32, tag="psD", name="psD")
                        for dtt in range(KT):
                            nc.tensor.matmul(
                                ps[:],
                                resT[:, dtt, nb * P : (nb + 1) * P],
                                wslab[:, dtt, dc * 512 : (dc + 1) * 512],
                                start=(dtt == 0),
                                stop=(dtt == KT - 1),
                            )
                        if D_EVICT_ACT:
                            nc.scalar.copy(osb[:, dc * 512 : (dc + 1) * 512], ps[:])
                        else:
                            nc.vector.tensor_copy(
                                osb[:, dc * 512 : (dc + 1) * 512], ps[:]
                            )
                    row0 = ch * NCH_C + nb * P
                    nc.sync.dma_start(out_d[:][row0 : row0 + P, :], osb[:])

    nc.compile()
    return nc


def _prep_inputs(text, context, We, W2):
    """Per-core host-side shard + transpose + fp32r pre-round."""
    we_r = r11(We)
    w2T_r = r11(W2.T)
    maps = []
    for b in range(B):
        maps.append(
            {
                "textT": r11(text[b].T),
                "ctxT": r11(context[b].T),
                "ctx": context[b].astype(ml_dtypes.bfloat16),
                "we": we_r,
                "w2T": w2T_r,
            }
        )
    return maps


def kernel(text, context, We, W2, _trace=False):
    from concourse.bass_utils import run_bass_kernel_spmd

    if "nc" not in _cache:
        _cache["nc"] = _build()
    nc = _cache["nc"]
    in_maps = _prep_inputs(
        np.asarray(text), np.asarray(context), np.asarray(We), np.asarray(W2)
    )
    res = run_bass_kernel_spmd(nc, in_maps, list(range(B)), trace=_trace)
    out = np.stack([res.results[c]["out"] for c in range(B)])
    if _trace:
        return out, res
    return out

